# revision 99
# baseline (speedup 1.0000x reference)
"""MHCLiteBlock Trainium2 kernel.

Data-parallel over T across 8 NeuronCores (1024 tokens/core); all params
replicated. Host ships x twice: token-major bf16 (xn) and feature-major
fp8e4m3 (xT, feeds only the W_all projection whose output is scaled by
alpha=0.01 before sigmoid/softmax, so fp8 precision is ample; W_all is
shipped fp8 with a x16 range shift undone in the coefficient stage).

Per core, per 128-token tile:
  1. DMA xn [128, 8192] bf16 + xT in 4 quarters [128, 16, 128] fp8.
  2. PE: proj[128, 32] = x^T-chunks (stationary) @ W_all cols (moving),
     accumulated over 64 K-chunks. ACT: ssq = sum(x^2) per token
     (4 Square ops with free-dim accumulate).
  3. Coefficient chain: rms = sqrt(mean+eps); scaled = proj*irms*alpha
     + b_all (one STT); sigmoid on ACT; exp via exp(v)=sig(v)/sig(-v)
     (keeps ACT on two tables: sqrt_and_friends / sigmoid_and_friends);
     soft permutation via perm-matrix matmul; normalize -> H coeffs.
  4. li = sum_m hpre_m*x_m: 16 diag matmuls on PE; ACT copies -> libf
     bf16; two half-row DMA-xbar transposes -> liT.
  5. diff = li @ (W_layer.T - I) + b_layer: 64 matmuls + K=1 ones-row
     bias matmul per 512-chunk; high-priority ACT copies -> diffbf bf16.
  6. Mix out_n = sum_m H[n,m] x_m + 2*hpost_n * diff:
     - stream 0 (all 4 streams on the last tile): PE diag matmuls into
       PSUM + one DVE scalar_tensor_tensor merge per 512-chunk.
     - streams 1-3: DVE tensor_scalar products (4x mode) + tensor_tensor
       adds (2x) in 1024-wide halves; 8 level-1 adds/tile on GPSIMD.
  7. Per-stream bf16 stores; host casts to fp32.

Scheduling: software-pipelined emission (stage A one tile ahead), xT
quarter prefetch depth 5, triple-buffered xn/outputs, the 8 MB W_layer
load deferred behind tile 0's inputs, high-priority on the coefficient
chain and PSUM-freeing copies. Cost-model exec: ~278.6 us/core (baseline
518.8 us).

Self-contained: hardcodes shapes; builds the Bass program once and
caches it.
"""

import sys

sys.path.insert(0, "/opt/trn_rl_repo")

from contextlib import ExitStack

import ml_dtypes
import numpy as np

import concourse.bass as bass
import concourse.mybir as mybir
import concourse.tile as tile
from concourse import bacc, bass_utils

F32 = mybir.dt.float32
BF16 = mybir.dt.bfloat16
FP8 = mybir.dt.float8e4
AF = mybir.ActivationFunctionType
ALU = mybir.AluOpType

T, N, C = 8192, 4, 2048
NCF = N * C  # 8192 flattened features
NFACT = 24
NCORES = 8
P = 128  # partitions / tokens per tile
NK = NCF // P  # 64 feature chunks
EPS = float(np.finfo(np.float32).eps)

# ---- mix assignment config ----
MIX_PE = (0,)  # streams mixed via PE diag matmuls + DVE STT merge
MIX_PE_LAST = (0, 1, 2, 3)  # last tile: PE is idle by then, DVE is the tail
# For DVE-mixed streams: which level-1 adds go to Pool (per stream index)
POOL_L1 = {1: (0, 1), 2: (0,), 3: (0,)}


def build_program(t_core: int, reps: int = 1, num_devices: int = NCORES,
                  with_bias: bool = True):
    nt = t_core // P
    nc = bacc.Bacc(
        "TRN2", target_bir_lowering=False, debug=False, num_devices=num_devices
    )

    xn_d = nc.dram_tensor("x", [t_core, NCF], BF16, kind="ExternalInput").ap()
    xt_d = nc.dram_tensor("xT", [nt * 4, P, NK // 4, P], FP8,
                          kind="ExternalInput").ap()
    wallt_d = nc.dram_tensor("wallt", [P, NK, 32], FP8, kind="ExternalInput").ap()
    wp_d = nc.dram_tensor("wp", [P, 16, C], BF16, kind="ExternalInput").ap()
    blayer_d = nc.dram_tensor("blayer", [1, C], BF16, kind="ExternalInput").ap()
    ones_d = nc.dram_tensor("ones1", [1, P], BF16, kind="ExternalInput").ap()
    perm_d = nc.dram_tensor("permaug", [NFACT, 17], F32, kind="ExternalInput").ap()
    ab_d = nc.dram_tensor("alphab", [2, 32], F32, kind="ExternalInput").ap()
    pc_d = nc.dram_tensor("polyc", [4, 32], F32, kind="ExternalInput").ap()
    idbf_d = nc.dram_tensor("idbf", [P, P], BF16, kind="ExternalInput").ap()
    idf32_d = nc.dram_tensor("idf32", [P, P], F32, kind="ExternalInput").ap()
    out_d = nc.dram_tensor("out", [t_core, NCF], BF16, kind="ExternalOutput").ap()

    with tile.TileContext(nc) as tc:
        _build_body(
            tc, nt, reps, xn_d, xt_d, wallt_d, wp_d, blayer_d, ones_d, perm_d,
            ab_d, pc_d, idbf_d, idf32_d, out_d, with_bias,
        )
    nc.compile()
    return nc


def _build_body(
    tc, nt, reps, xn_d, xt_d, wallt_d, wp_d, blayer_d, ones_d, perm_d, ab_d,
    pc_d, idbf_d, idf32_d, out_d, with_bias,
):
    nc = tc.nc
    with ExitStack() as ctx:
        singles = ctx.enter_context(tc.tile_pool(name="singles", bufs=1))
        xnp = ctx.enter_context(tc.tile_pool(name="xnp", bufs=3))
        xtp = ctx.enter_context(tc.tile_pool(name="xtp", bufs=5))
        smalls = ctx.enter_context(tc.tile_pool(name="smalls", bufs=3))
        diagp = ctx.enter_context(tc.tile_pool(name="diagp", bufs=3))
        diagl = ctx.enter_context(tc.tile_pool(name="diagl", bufs=1))
        lip = ctx.enter_context(tc.tile_pool(name="lip", bufs=2))
        dfp = ctx.enter_context(tc.tile_pool(name="dfp", bufs=2))
        sqp = ctx.enter_context(tc.tile_pool(name="sqp", bufs=1))
        mxp = ctx.enter_context(tc.tile_pool(name="mxp", bufs=2))
        outp = ctx.enter_context(tc.tile_pool(name="outp", bufs=3))
        ps_small = ctx.enter_context(
            tc.tile_pool(name="ps_small", bufs=2, space="PSUM")
        )
        ps_li = ctx.enter_context(tc.tile_pool(name="ps_li", bufs=1, space="PSUM"))
        ps_diff = ctx.enter_context(
            tc.tile_pool(name="ps_diff", bufs=2, space="PSUM")
        )
        ps_mix = ctx.enter_context(tc.tile_pool(name="ps_mix", bufs=3, space="PSUM"))

        # ---- small one-time parameter loads (beat the tile loads' prio) ----
        ctx_hp0 = tc.high_priority()
        ctx_hp0.__enter__()
        walls = singles.tile([P, NK, 32], FP8)
        nc.sync.dma_start(out=walls[:], in_=wallt_d[:])
        wp_s = singles.tile([P, 16, C], BF16)
        blayer_s = singles.tile([1, C], BF16)
        nc.sync.dma_start(out=blayer_s[:], in_=blayer_d[:])
        ones_s = singles.tile([1, P], BF16)
        nc.sync.dma_start(out=ones_s[:], in_=ones_d[:])
        perm_s = singles.tile([NFACT, 17], F32)
        nc.sync.dma_start(out=perm_s[:], in_=perm_d[:])
        idbf_s = singles.tile([P, P], BF16)
        nc.sync.dma_start(out=idbf_s[:], in_=idbf_d[:])
        idf32_s = singles.tile([P, P], F32)
        nc.sync.dma_start(out=idf32_s[:], in_=idf32_d[:])
        alpha_b = singles.tile([P, 32], F32)
        nc.gpsimd.dma_start(
            out=alpha_b[:],
            in_=bass.AP(tensor=ab_d.tensor, offset=ab_d.offset,
                        ap=[[0, P], [1, 32]]),
        )
        bias_b = singles.tile([P, 32], F32)
        nc.gpsimd.dma_start(
            out=bias_b[:],
            in_=bass.AP(tensor=ab_d.tensor, offset=ab_d.offset + 32,
                        ap=[[0, P], [1, 32]]),
        )
        eps_t = singles.tile([P, 1], F32)
        nc.vector.memset(eps_t[:], EPS)
        warm = singles.tile([P, 1], F32)
        nc.scalar.activation(out=warm[:], in_=eps_t[:], func=AF.Sqrt)
        ctx_hp0.__exit__(None, None, None)

        def stage_a(t):
            """Loads + proj + stats + coefficients + diags."""
            rows = slice(t * P, (t + 1) * P)
            st = {"rows": rows, "t": t}

            # proj on PE: projT[32, P] accumulated over 64 K-chunks;
            # xT streamed in quarters to bound SBUF. Loads first + high
            # priority so they preempt elastic output stores on the DMA.
            NQ = NK // 4
            xn = xnp.tile([P, NCF], BF16, tag="xn", name=f"xn{t}")
            if t == 0:
                nc.sync.dma_start(out=xn[:], in_=xn_d[rows, :])
            xTs = []
            for qq in range(4):
                xT = xtp.tile([P, NQ, P], FP8, tag="xT", name=f"xT{t}_{qq}")
                nc.sync.dma_start(out=xT[:], in_=xt_d[t * 4 + qq])
                xTs.append(xT)
            if t > 0:
                nc.sync.dma_start(out=xn[:], in_=xn_d[rows, :])
            # proj direct in token-major (stationary xT chunk, moving walls)
            proj_p = ps_small.tile([P, 32], F32, tag="pssmall", name=f"prp{t}")
            for qq in range(4):
                for kk in range(NQ):
                    k = qq * NQ + kk
                    nc.tensor.matmul(
                        proj_p[:], xTs[qq][:, kk, :], walls[:, k, :],
                        start=(k == 0), stop=(k == NK - 1),
                    )

            # ssq on ACT: Square with free-dim accumulate, 4 chunks
            ssqp = smalls.tile([P, N], F32, tag="ssqp", name=f"ssqp{t}")
            for m in range(N):
                sq = sqp.tile([P, C], BF16, tag="sq", name=f"sq{t}_{m}")
                nc.scalar.activation(
                    out=sq[:], in_=xn[:, m * C:(m + 1) * C], func=AF.Square,
                    accum_out=ssqp[:, m:m + 1],
                )
            ctx_hp = tc.high_priority()
            ctx_hp.__enter__()
            ssq = smalls.tile([P, 1], F32, tag="ssq", name=f"ssq{t}")
            nc.vector.tensor_reduce(
                out=ssq[:], in_=ssqp[:], axis=mybir.AxisListType.X, op=ALU.add
            )
            rms = smalls.tile([P, 1], F32, tag="rms", name=f"rms{t}")
            nc.scalar.activation(
                out=rms[:], in_=ssq[:], func=AF.Sqrt, bias=eps_t[:],
                scale=1.0 / NCF,
            )
            irms = smalls.tile([P, 1], F32, tag="irms", name=f"irms{t}")
            nc.vector.reciprocal(out=irms[:], in_=rms[:])

            # walls carry x16 (fp8 range); alpha_b = alpha/16 per column
            scaled = smalls.tile([P, 32], F32, tag="scaled", name=f"scl{t}")
            nc.vector.scalar_tensor_tensor(
                out=scaled[:], in0=proj_p[:], scalar=irms[:], in1=alpha_b[:],
                op0=ALU.mult, op1=ALU.mult,
            )
            nc.vector.tensor_add(scaled[:], scaled[:], bias_b[:])

            # acts: cols 0:4 sigmoid (h_pre), 4:8 2*sigmoid (2*h_post),
            # 8:32 exp (softmax numerator, via exp(v) = sig(v)/sig(-v)).
            acts = smalls.tile([P, 32], F32, tag="acts", name=f"acts{t}")
            nc.scalar.activation(out=acts[:, 0:8], in_=scaled[:, 0:8],
                                 func=AF.Sigmoid)
            nc.vector.tensor_scalar_mul(acts[:, 4:8], acts[:, 4:8], 2.0)
            sigp = smalls.tile([P, NFACT], F32, tag="sigp", name=f"sigp{t}")
            nc.scalar.activation(out=sigp[:], in_=scaled[:, 8:32],
                                 func=AF.Sigmoid)
            sign = smalls.tile([P, NFACT], F32, tag="sign", name=f"sign{t}")
            nc.scalar.activation(
                out=sign[:], in_=scaled[:, 8:32], func=AF.Sigmoid, scale=-1.0
            )
            signr = smalls.tile([P, NFACT], F32, tag="signr", name=f"signr{t}")
            nc.vector.reciprocal(out=signr[:], in_=sign[:])
            exps = smalls.tile([P, NFACT], F32, tag="exps", name=f"exps{t}")
            nc.vector.tensor_mul(exps[:], sigp[:], signr[:])
            hps = acts

            expsT_p = ps_small.tile([NFACT, P], F32, tag="pssmall", name=f"exT{t}")
            nc.tensor.transpose(expsT_p[:], exps[:], idf32_s[:])
            expsT_s = smalls.tile([NFACT, P], F32, tag="expsT_s", name=f"exs{t}")
            nc.scalar.activation(out=expsT_s[:], in_=expsT_p[:], func=AF.Copy)

            haug_p = ps_small.tile([P, 17], F32, tag="pssmall", name=f"hgp{t}")
            nc.tensor.matmul(
                haug_p[:], expsT_s[:], perm_s[:], start=True, stop=True
            )
            hd = smalls.tile([P, 17], F32, tag="hd", name=f"hd{t}")
            nc.scalar.activation(out=hd[:], in_=haug_p[:], func=AF.Copy)

            dinv = smalls.tile([P, 1], F32, tag="dinv", name=f"dinv{t}")
            nc.vector.reciprocal(out=dinv[:], in_=hd[:, 16:17])

            # coeffs: normalized H (col 4m+n = H[n,m]); 2*h_post is acts[:,4:8]
            coeffs = smalls.tile([P, 16], F32, tag="coeffs", name=f"co{t}")
            nc.vector.tensor_scalar_mul(coeffs[:, 0:16], hd[:, 0:16], dinv[:])

            # diags: j<4 -> hpre_j (li)
            diags = diagp.tile([P, N, P], BF16, tag="diags", name=f"dg{t}")
            for m in range(N):
                nc.vector.tensor_scalar_mul(
                    diags[:, m, :], idbf_s[:], hps[:, m:m + 1]
                )
            ctx_hp.__exit__(None, None, None)

            st["xn"] = xn
            st["coeffs"] = coeffs
            st["acts"] = acts
            st["diags"] = diags
            return st

        def stage_b(st):
            """li -> liT -> diff -> diffbf."""
            xn = st["xn"]
            diags = st["diags"]
            t = st["t"]

            libf = lip.tile([P, C], BF16, tag="libf", name=f"libf{t}")
            liT = lip.tile([P, 16, P], BF16, tag="liT", name=f"liT{t}")
            with tc.high_priority():
                for q in range(4):
                    cs = slice(q * 512, (q + 1) * 512)
                    li_p = ps_li.tile([P, 512], F32, tag="li", name=f"lip{t}_{q}")
                    for m in range(N):
                        nc.tensor.matmul(
                            li_p[:], diags[:, m, :],
                            xn[:, m * C + q * 512: m * C + (q + 1) * 512],
                            start=(m == 0), stop=(m == 3),
                        )
                    nc.scalar.activation(out=libf[:, cs], in_=li_p[:],
                                         func=AF.Copy)
                    if q % 2 == 1:
                        hs = slice((q // 2) * 1024, (q // 2 + 1) * 1024)
                        nc.sync.dma_start_transpose(
                            out=liT[:, (q // 2) * 8:(q // 2 + 1) * 8, :],
                            in_=libf[:, hs],
                        )

            diffbf = dfp.tile([P, C], BF16, tag="diffbf", name=f"dfb{t}")
            for q in range(4):
                cs = slice(q * 512, (q + 1) * 512)
                diff_p = ps_diff.tile([P, 512], F32, tag="diff", name=f"dfp{t}_{q}")
                for k in range(16):
                    nc.tensor.matmul(
                        diff_p[:], liT[:, k, :], wp_s[:, k, cs],
                        start=(k == 0), stop=(not with_bias and k == 15),
                    )
                if with_bias:
                    # bias fold: ones column (K=1) x b_layer row chunk
                    nc.tensor.matmul(
                        diff_p[:], ones_s[:], blayer_s[:, cs],
                        start=False, stop=True,
                    )
                with tc.high_priority():
                    nc.scalar.activation(out=diffbf[:, cs], in_=diff_p[:],
                                         func=AF.Copy)

            st["diffbf"] = diffbf
            return st

        def stage_c(st):
            """Mix + store."""
            xn = st["xn"]
            diags = st["diags"]
            coeffs = st["coeffs"]
            diffbf = st["diffbf"]
            rows = st["rows"]
            t = st["t"]

            mix_pe = st["mix_pe"]
            acts = st["acts"]
            tag_sfx = "L" if len(mix_pe) == N else ""
            mdiags = None
            if mix_pe:
                mpool = diagl if tag_sfx else diagp
                mdiags = mpool.tile([P, 4 * len(mix_pe), P], BF16,
                                    tag=f"mdiags{tag_sfx}", name=f"mdg{t}")
                for i, n in enumerate(mix_pe):
                    for m in range(N):
                        nc.vector.tensor_scalar_mul(
                            mdiags[:, 4 * i + m, :], idbf_s[:],
                            coeffs[:, 4 * m + n:4 * m + n + 1],
                        )

            for i, n in enumerate(mix_pe):
                outsb = outp.tile([P, C], BF16, tag="outsb", name=f"ou{t}_{n}")
                for cc in range(4):
                    cs = slice(cc * 512, (cc + 1) * 512)
                    mix_p = ps_mix.tile([P, 512], F32, tag="mix",
                                        name=f"mx{t}_{n}_{cc}")
                    for src in range(N):
                        nc.tensor.matmul(
                            mix_p[:], mdiags[:, 4 * i + src, :],
                            xn[:, src * C + cc * 512: src * C + (cc + 1) * 512],
                            start=(src == 0), stop=(src == 3),
                        )
                    nc.vector.scalar_tensor_tensor(
                        out=outsb[:, cs],
                        in0=diffbf[:, cs],
                        scalar=acts[:, 4 + n:5 + n], in1=mix_p[:],
                        op0=ALU.mult, op1=ALU.add,
                    )
                nc.sync.dma_start(
                    out=out_d[rows, n * C:(n + 1) * C], in_=outsb[:]
                )

            H = C // 2
            for n in range(N):
                if n in mix_pe:
                    continue
                outsb = outp.tile([P, C], BF16, tag="outsb", name=f"ou{t}_{n}")
                for h in range(2):
                    hs = slice(h * H, (h + 1) * H)
                    ts_ = [
                        mxp.tile([P, H], BF16, tag=f"mt{j}",
                                 name=f"mt{t}_{n}_{h}_{j}")
                        for j in range(4)
                    ]
                    td = mxp.tile([P, H], BF16, tag="mtd", name=f"mtd{t}_{n}_{h}")
                    for m in range(N):
                        nc.vector.tensor_scalar_mul(
                            ts_[m][:], xn[:, m * C + h * H: m * C + (h + 1) * H],
                            coeffs[:, 4 * m + n:4 * m + n + 1],
                        )
                    nc.vector.tensor_scalar_mul(
                        td[:], diffbf[:, hs], acts[:, 4 + n:5 + n]
                    )
                    l1 = POOL_L1.get(n, ())
                    eng0 = nc.gpsimd if 0 in l1 else nc.vector
                    eng1 = nc.gpsimd if 1 in l1 else nc.vector
                    eng0.tensor_tensor(out=ts_[0][:], in0=ts_[0][:],
                                       in1=ts_[1][:], op=ALU.add)
                    eng1.tensor_tensor(out=ts_[2][:], in0=ts_[2][:],
                                       in1=ts_[3][:], op=ALU.add)
                    nc.vector.tensor_tensor(out=ts_[0][:], in0=ts_[0][:],
                                            in1=ts_[2][:], op=ALU.add)
                    nc.vector.tensor_tensor(
                        out=outsb[:, h * H:(h + 1) * H],
                        in0=ts_[0][:], in1=td[:], op=ALU.add,
                    )
                nc.sync.dma_start(
                    out=out_d[rows, n * C:(n + 1) * C], in_=outsb[:]
                )

        # ---- software-pipelined emission: stage A runs one tile ahead ----
        pending = None
        wp_chunks = 0
        for rep in range(reps):
            for t in range(nt):
                st = stage_a(t)
                st["mix_pe"] = (
                    MIX_PE_LAST if (rep == reps - 1 and t == nt - 1) else MIX_PE
                )
                # defer the big weight load until after tile 0's loads
                # (single DMA; emitted before its first consumer stage_b(0))
                if rep == 0 and wp_chunks == 0:
                    nc.sync.dma_start(out=wp_s[:], in_=wp_d[:])
                    wp_chunks = 4
                if pending is not None:
                    stage_b(pending)
                    stage_c(pending)
                pending = st
        stage_b(pending)
        stage_c(pending)


def prep_params(inputs):
    """Host-side parameter preprocessing shared by all cores."""
    bf = ml_dtypes.bfloat16
    W_all = np.asarray(inputs["W_all"], np.float32)
    W_layer = np.asarray(inputs["W_layer"], np.float32)
    b_all = np.asarray(inputs["b_all"], np.float32)
    b_layer = np.asarray(inputs["b_layer"], np.float32)
    perm_mat = np.asarray(inputs["perm_mat"], np.float32)
    a_pre = float(np.asarray(inputs["alpha_pre"]).reshape(-1)[0])
    a_post = float(np.asarray(inputs["alpha_post"]).reshape(-1)[0])
    a_res = float(np.asarray(inputs["alpha_res"]).reshape(-1)[0])

    f8 = ml_dtypes.float8_e4m3
    wallt = np.ascontiguousarray(
        (W_all * 16.0).T.astype(f8).reshape(NK, P, 32).transpose(1, 0, 2)
    )
    wp = (np.ascontiguousarray(W_layer.T) - np.eye(C, dtype=np.float32))
    wp = np.ascontiguousarray(wp.astype(bf).reshape(16, P, C).transpose(1, 0, 2))
    blayer = b_layer.astype(bf).reshape(1, C)
    ones1 = np.ones((1, P), dtype=bf)
    # perm_aug columns m-major: col 4m+n = perm_mat[:, n*4+m]; col 16 = 1
    perm_aug = np.zeros((NFACT, 17), np.float32)
    perm_aug[:, :16] = perm_mat.reshape(NFACT, N, N).transpose(0, 2, 1).reshape(
        NFACT, 16
    )
    perm_aug[:, 16] = 1.0
    alphab = np.zeros((2, 32), np.float32)
    alphab[0, 0:4] = a_pre / 16.0
    alphab[0, 4:8] = a_post / 16.0
    alphab[0, 8:32] = a_res / 16.0
    alphab[1, 0:4] = b_all[0:4]
    alphab[1, 4:8] = b_all[4:8]
    alphab[1, 8:32] = b_all[8:32]
    # cubic activation polys: cols 0:4 sigmoid, 4:8 2*sigmoid, 8:32 exp
    polyc = np.zeros((4, 32), np.float32)
    polyc[0, 0:4], polyc[1, 0:4], polyc[2, 0:4], polyc[3, 0:4] = \
        0.5, 0.25, 0.0, -1.0 / 48.0
    polyc[0, 4:8], polyc[1, 4:8], polyc[2, 4:8], polyc[3, 4:8] = \
        1.0, 0.5, 0.0, -1.0 / 24.0
    polyc[0, 8:32], polyc[1, 8:32], polyc[2, 8:32], polyc[3, 8:32] = \
        1.0, 1.0, 0.5, 1.0 / 6.0
    idbf = np.eye(P, dtype=np.float32).astype(bf)
    idf32 = np.eye(P, dtype=np.float32)
    return {
        "wallt": wallt, "wp": wp, "blayer": blayer, "ones1": ones1,
        "permaug": perm_aug, "alphab": alphab, "polyc": polyc,
        "idbf": idbf, "idf32": idf32,
    }


_PROGRAM_CACHE = {}


def get_program(t_core, with_bias=True):
    key = (t_core, with_bias)
    if key not in _PROGRAM_CACHE:
        _PROGRAM_CACHE[key] = build_program(t_core, with_bias=with_bias)
    return _PROGRAM_CACHE[key]


def run(inputs, trace=False):
    bf = ml_dtypes.bfloat16
    x = np.asarray(inputs["x_streams"], np.float32).reshape(T, NCF).astype(bf)
    params = prep_params(inputs)
    t_core = T // NCORES
    nt = t_core // P
    nc = get_program(t_core, True)
    in_maps = []
    for c in range(NCORES):
        m = dict(params)
        xc = x[c * t_core:(c + 1) * t_core]
        m["x"] = np.ascontiguousarray(xc)
        m["xT"] = np.ascontiguousarray(
            xc.reshape(nt, P, 4, NK // 4, P).transpose(0, 2, 4, 3, 1)
        ).astype(ml_dtypes.float8_e4m3).reshape(nt * 4, P, NK // 4, P)
        in_maps.append(m)
    res = bass_utils.run_bass_kernel_spmd(
        nc, in_maps, core_ids=list(range(NCORES)), trace=trace
    )
    out = np.concatenate([r["out"] for r in res.results], axis=0)
    return out.reshape(T, N, C).astype(np.float32), res


def kernel(**inputs) -> np.ndarray:
    out, _ = run(inputs)
    return out


# revision 112
# speedup vs baseline: 1.0094x; 1.0094x over previous
"""MHCLiteBlock Trainium2 kernel.

Data-parallel over T across 8 NeuronCores (1024 tokens/core); all params
replicated. Host ships x twice: token-major bf16 (xn) and feature-major
fp8e4m3 (xT, feeds only the W_all projection whose output is scaled by
alpha=0.01 before sigmoid/softmax, so fp8 precision is ample; W_all is
shipped fp8 with a x16 range shift undone in the coefficient stage).

Per core, per 128-token tile:
  1. DMA xn [128, 8192] bf16 + xT in 4 quarters [128, 16, 128] fp8.
  2. PE: proj[128, 32] = x^T-chunks (stationary) @ W_all cols (moving),
     accumulated over 64 K-chunks. ACT: ssq = sum(x^2) per token
     (4 Square ops with free-dim accumulate).
  3. Coefficient chain: rms = sqrt(mean+eps); scaled = proj*irms*alpha
     + b_all (one STT); sigmoid on ACT; exp via exp(v)=sig(v)/sig(-v)
     (keeps ACT on two tables: sqrt_and_friends / sigmoid_and_friends);
     soft permutation via perm-matrix matmul; normalize -> H coeffs.
  4. li = sum_m hpre_m*x_m: 16 diag matmuls on PE; ACT copies -> libf
     bf16; two half-row DMA-xbar transposes -> liT.
  5. diff = li @ (W_layer.T - I) + b_layer: 64 matmuls + K=1 ones-row
     bias matmul per 512-chunk; high-priority ACT copies -> diffbf bf16.
  6. Mix out_n = sum_m H[n,m] x_m + 2*hpost_n * diff:
     - stream 0 (all 4 streams on the last tile): PE diag matmuls into
       PSUM + one DVE scalar_tensor_tensor merge per 512-chunk.
     - streams 1-3: DVE tensor_scalar products (4x mode) + tensor_tensor
       adds (2x) in 1024-wide halves; 8 level-1 adds/tile on GPSIMD.
  7. Per-stream bf16 stores; host casts to fp32.

Scheduling: software-pipelined emission (stage A one tile ahead), xT
quarter prefetch depth 5, triple-buffered xn/outputs, the 8 MB W_layer
load deferred behind tile 0's inputs, high-priority on the coefficient
chain and PSUM-freeing copies. Cost-model exec: ~276.0 us/core (baseline
518.8 us).

Self-contained: hardcodes shapes; builds the Bass program once and
caches it.
"""

import sys

sys.path.insert(0, "/opt/trn_rl_repo")

from contextlib import ExitStack

import ml_dtypes
import numpy as np

import concourse.bass as bass
import concourse.mybir as mybir
import concourse.tile as tile
from concourse import bacc, bass_utils

F32 = mybir.dt.float32
BF16 = mybir.dt.bfloat16
FP8 = mybir.dt.float8e4
AF = mybir.ActivationFunctionType
ALU = mybir.AluOpType

T, N, C = 8192, 4, 2048
NCF = N * C  # 8192 flattened features
NFACT = 24
NCORES = 8
P = 128  # partitions / tokens per tile
NK = NCF // P  # 64 feature chunks
EPS = float(np.finfo(np.float32).eps)

# ---- mix assignment config ----
MIX_PE = (0,)  # streams mixed via PE diag matmuls + DVE STT merge
MIX_PE_LAST = (0, 1, 2, 3)  # last tile: PE is idle by then, DVE is the tail
# For DVE-mixed streams: which level-1 adds go to Pool (per stream index)
POOL_L1 = {1: (0, 1), 2: (0,), 3: (0,)}


def build_program(t_core: int, reps: int = 1, num_devices: int = NCORES,
                  with_bias: bool = True):
    nt = t_core // P
    nc = bacc.Bacc(
        "TRN2", target_bir_lowering=False, debug=False, num_devices=num_devices
    )

    xn_d = nc.dram_tensor("x", [t_core, NCF], BF16, kind="ExternalInput").ap()
    xt_d = nc.dram_tensor("xT", [nt * 4, P, NK // 4, P], FP8,
                          kind="ExternalInput").ap()
    wallt_d = nc.dram_tensor("wallt", [P, NK, 32], FP8, kind="ExternalInput").ap()
    wp_d = nc.dram_tensor("wp", [P, 16, C], BF16, kind="ExternalInput").ap()
    blayer_d = nc.dram_tensor("blayer", [1, C], BF16, kind="ExternalInput").ap()
    ones_d = nc.dram_tensor("ones1", [1, P], BF16, kind="ExternalInput").ap()
    perm_d = nc.dram_tensor("permaug", [NFACT, 17], F32, kind="ExternalInput").ap()
    ab_d = nc.dram_tensor("alphab", [2, 32], F32, kind="ExternalInput").ap()
    pc_d = nc.dram_tensor("polyc", [4, 32], F32, kind="ExternalInput").ap()
    idbf_d = nc.dram_tensor("idbf", [P, P], BF16, kind="ExternalInput").ap()
    idf32_d = nc.dram_tensor("idf32", [P, P], F32, kind="ExternalInput").ap()
    out_d = nc.dram_tensor("out", [t_core, NCF], BF16, kind="ExternalOutput").ap()

    with tile.TileContext(nc) as tc:
        _build_body(
            tc, nt, reps, xn_d, xt_d, wallt_d, wp_d, blayer_d, ones_d, perm_d,
            ab_d, pc_d, idbf_d, idf32_d, out_d, with_bias,
        )
    nc.compile()
    return nc


def _build_body(
    tc, nt, reps, xn_d, xt_d, wallt_d, wp_d, blayer_d, ones_d, perm_d, ab_d,
    pc_d, idbf_d, idf32_d, out_d, with_bias,
):
    nc = tc.nc
    with ExitStack() as ctx:
        singles = ctx.enter_context(tc.tile_pool(name="singles", bufs=1))
        xnp = ctx.enter_context(tc.tile_pool(name="xnp", bufs=3))
        xtp = ctx.enter_context(tc.tile_pool(name="xtp", bufs=5))
        smalls = ctx.enter_context(tc.tile_pool(name="smalls", bufs=3))
        diagp = ctx.enter_context(tc.tile_pool(name="diagp", bufs=3))
        diagl = ctx.enter_context(tc.tile_pool(name="diagl", bufs=1))
        lip = ctx.enter_context(tc.tile_pool(name="lip", bufs=2))
        dfp = ctx.enter_context(tc.tile_pool(name="dfp", bufs=2))
        sqp = ctx.enter_context(tc.tile_pool(name="sqp", bufs=1))
        mxp = ctx.enter_context(tc.tile_pool(name="mxp", bufs=2))
        outp = ctx.enter_context(tc.tile_pool(name="outp", bufs=3))
        ps_small = ctx.enter_context(
            tc.tile_pool(name="ps_small", bufs=2, space="PSUM")
        )
        ps_li = ctx.enter_context(tc.tile_pool(name="ps_li", bufs=1, space="PSUM"))
        ps_diff = ctx.enter_context(
            tc.tile_pool(name="ps_diff", bufs=2, space="PSUM")
        )
        ps_mix = ctx.enter_context(tc.tile_pool(name="ps_mix", bufs=3, space="PSUM"))

        # ---- small one-time parameter loads (beat the tile loads' prio) ----
        ctx_hp0 = tc.high_priority()
        ctx_hp0.__enter__()
        walls = singles.tile([P, NK, 32], FP8)
        nc.sync.dma_start(out=walls[:], in_=wallt_d[:])
        ctx_hp0.__exit__(None, None, None)
        wp_s = singles.tile([P, 16, C], BF16)
        blayer_s = singles.tile([1, C], BF16)
        nc.sync.dma_start(out=blayer_s[:], in_=blayer_d[:])
        ones_s = singles.tile([1, P], BF16)
        nc.sync.dma_start(out=ones_s[:], in_=ones_d[:])
        perm_s = singles.tile([NFACT, 17], F32)
        nc.sync.dma_start(out=perm_s[:], in_=perm_d[:])
        idbf_s = singles.tile([P, P], BF16)
        nc.sync.dma_start(out=idbf_s[:], in_=idbf_d[:])
        idf32_s = singles.tile([P, P], F32)
        nc.sync.dma_start(out=idf32_s[:], in_=idf32_d[:])
        alpha_b = singles.tile([P, 32], F32)
        nc.gpsimd.dma_start(
            out=alpha_b[:],
            in_=bass.AP(tensor=ab_d.tensor, offset=ab_d.offset,
                        ap=[[0, P], [1, 32]]),
        )
        bias_b = singles.tile([P, 32], F32)
        nc.gpsimd.dma_start(
            out=bias_b[:],
            in_=bass.AP(tensor=ab_d.tensor, offset=ab_d.offset + 32,
                        ap=[[0, P], [1, 32]]),
        )
        eps_t = singles.tile([P, 1], F32)
        nc.vector.memset(eps_t[:], EPS)
        warm = singles.tile([P, 1], F32)
        nc.scalar.activation(out=warm[:], in_=eps_t[:], func=AF.Sqrt)

        def stage_a(t):
            """Loads + proj + stats + coefficients + diags."""
            rows = slice(t * P, (t + 1) * P)
            st = {"rows": rows, "t": t}

            # proj on PE: projT[32, P] accumulated over 64 K-chunks;
            # xT streamed in quarters to bound SBUF. Loads first + high
            # priority so they preempt elastic output stores on the DMA.
            NQ = NK // 4
            xn = xnp.tile([P, NCF], BF16, tag="xn", name=f"xn{t}")
            xTs = []
            if t == 0:
                with tc.high_priority(offset=None):
                    for qq in range(4):
                        xT = xtp.tile([P, NQ, P], FP8, tag="xT",
                                      name=f"xT{t}_{qq}")
                        nc.sync.dma_start(out=xT[:], in_=xt_d[t * 4 + qq])
                        xTs.append(xT)
                    nc.sync.dma_start(out=xn[:], in_=xn_d[rows, :])
            else:
                for qq in range(4):
                    xT = xtp.tile([P, NQ, P], FP8, tag="xT", name=f"xT{t}_{qq}")
                    nc.sync.dma_start(out=xT[:], in_=xt_d[t * 4 + qq])
                    xTs.append(xT)
                nc.sync.dma_start(out=xn[:], in_=xn_d[rows, :])
            # proj direct in token-major (stationary xT chunk, moving walls)
            proj_p = ps_small.tile([P, 32], F32, tag="pssmall", name=f"prp{t}")
            for qq in range(4):
                for kk in range(NQ):
                    k = qq * NQ + kk
                    nc.tensor.matmul(
                        proj_p[:], xTs[qq][:, kk, :], walls[:, k, :],
                        start=(k == 0), stop=(k == NK - 1),
                    )

            # ssq on ACT: Square with free-dim accumulate, 4 chunks
            ssqp = smalls.tile([P, N], F32, tag="ssqp", name=f"ssqp{t}")
            for m in range(N):
                sq = sqp.tile([P, C], BF16, tag="sq", name=f"sq{t}_{m}")
                nc.scalar.activation(
                    out=sq[:], in_=xn[:, m * C:(m + 1) * C], func=AF.Square,
                    accum_out=ssqp[:, m:m + 1],
                )
            ctx_hp = tc.high_priority()
            ctx_hp.__enter__()
            ssq = smalls.tile([P, 1], F32, tag="ssq", name=f"ssq{t}")
            nc.vector.tensor_reduce(
                out=ssq[:], in_=ssqp[:], axis=mybir.AxisListType.X, op=ALU.add
            )
            rms = smalls.tile([P, 1], F32, tag="rms", name=f"rms{t}")
            nc.scalar.activation(
                out=rms[:], in_=ssq[:], func=AF.Sqrt, bias=eps_t[:],
                scale=1.0 / NCF,
            )
            irms = smalls.tile([P, 1], F32, tag="irms", name=f"irms{t}")
            nc.vector.reciprocal(out=irms[:], in_=rms[:])

            # walls carry x16 (fp8 range); alpha_b = alpha/16 per column
            scaled = smalls.tile([P, 32], F32, tag="scaled", name=f"scl{t}")
            nc.vector.scalar_tensor_tensor(
                out=scaled[:], in0=proj_p[:], scalar=irms[:], in1=alpha_b[:],
                op0=ALU.mult, op1=ALU.mult,
            )
            nc.vector.tensor_add(scaled[:], scaled[:], bias_b[:])

            # acts: cols 0:4 sigmoid (h_pre), 4:8 2*sigmoid (2*h_post),
            # 8:32 exp (softmax numerator, via exp(v) = sig(v)/sig(-v)).
            acts = smalls.tile([P, 32], F32, tag="acts", name=f"acts{t}")
            nc.scalar.activation(out=acts[:, 0:8], in_=scaled[:, 0:8],
                                 func=AF.Sigmoid)
            nc.vector.tensor_scalar_mul(acts[:, 4:8], acts[:, 4:8], 2.0)
            sigp = smalls.tile([P, NFACT], F32, tag="sigp", name=f"sigp{t}")
            nc.scalar.activation(out=sigp[:], in_=scaled[:, 8:32],
                                 func=AF.Sigmoid)
            sign = smalls.tile([P, NFACT], F32, tag="sign", name=f"sign{t}")
            nc.scalar.activation(
                out=sign[:], in_=scaled[:, 8:32], func=AF.Sigmoid, scale=-1.0
            )
            signr = smalls.tile([P, NFACT], F32, tag="signr", name=f"signr{t}")
            nc.vector.reciprocal(out=signr[:], in_=sign[:])
            exps = smalls.tile([P, NFACT], F32, tag="exps", name=f"exps{t}")
            nc.vector.tensor_mul(exps[:], sigp[:], signr[:])
            hps = acts

            expsT_p = ps_small.tile([NFACT, P], F32, tag="pssmall", name=f"exT{t}")
            nc.tensor.transpose(expsT_p[:], exps[:], idf32_s[:])
            expsT_s = smalls.tile([NFACT, P], F32, tag="expsT_s", name=f"exs{t}")
            nc.scalar.activation(out=expsT_s[:], in_=expsT_p[:], func=AF.Copy)

            haug_p = ps_small.tile([P, 17], F32, tag="pssmall", name=f"hgp{t}")
            nc.tensor.matmul(
                haug_p[:], expsT_s[:], perm_s[:], start=True, stop=True
            )
            hd = smalls.tile([P, 17], F32, tag="hd", name=f"hd{t}")
            nc.scalar.activation(out=hd[:], in_=haug_p[:], func=AF.Copy)

            dinv = smalls.tile([P, 1], F32, tag="dinv", name=f"dinv{t}")
            nc.vector.reciprocal(out=dinv[:], in_=hd[:, 16:17])

            # coeffs: normalized H (col 4m+n = H[n,m]); 2*h_post is acts[:,4:8]
            coeffs = smalls.tile([P, 16], F32, tag="coeffs", name=f"co{t}")
            nc.vector.tensor_scalar_mul(coeffs[:, 0:16], hd[:, 0:16], dinv[:])

            # diags: j<4 -> hpre_j (li)
            diags = diagp.tile([P, N, P], BF16, tag="diags", name=f"dg{t}")
            for m in range(N):
                nc.vector.tensor_scalar_mul(
                    diags[:, m, :], idbf_s[:], hps[:, m:m + 1]
                )
            ctx_hp.__exit__(None, None, None)

            st["xn"] = xn
            st["coeffs"] = coeffs
            st["acts"] = acts
            st["diags"] = diags
            return st

        def stage_b(st):
            """li -> liT -> diff -> diffbf."""
            xn = st["xn"]
            diags = st["diags"]
            t = st["t"]

            libf = lip.tile([P, C], BF16, tag="libf", name=f"libf{t}")
            liT = lip.tile([P, 16, P], BF16, tag="liT", name=f"liT{t}")
            with tc.high_priority():
                for q in range(4):
                    cs = slice(q * 512, (q + 1) * 512)
                    li_p = ps_li.tile([P, 512], F32, tag="li", name=f"lip{t}_{q}")
                    for m in range(N):
                        nc.tensor.matmul(
                            li_p[:], diags[:, m, :],
                            xn[:, m * C + q * 512: m * C + (q + 1) * 512],
                            start=(m == 0), stop=(m == 3),
                        )
                    nc.scalar.activation(out=libf[:, cs], in_=li_p[:],
                                         func=AF.Copy)
                    if q % 2 == 1:
                        hs = slice((q // 2) * 1024, (q // 2 + 1) * 1024)
                        nc.sync.dma_start_transpose(
                            out=liT[:, (q // 2) * 8:(q // 2 + 1) * 8, :],
                            in_=libf[:, hs],
                        )

            diffbf = dfp.tile([P, C], BF16, tag="diffbf", name=f"dfb{t}")
            for q in range(4):
                cs = slice(q * 512, (q + 1) * 512)
                diff_p = ps_diff.tile([P, 512], F32, tag="diff", name=f"dfp{t}_{q}")
                for k in range(16):
                    nc.tensor.matmul(
                        diff_p[:], liT[:, k, :], wp_s[:, k, cs],
                        start=(k == 0), stop=(not with_bias and k == 15),
                    )
                if with_bias:
                    # bias fold: ones column (K=1) x b_layer row chunk
                    nc.tensor.matmul(
                        diff_p[:], ones_s[:], blayer_s[:, cs],
                        start=False, stop=True,
                    )
                with tc.high_priority():
                    nc.scalar.activation(out=diffbf[:, cs], in_=diff_p[:],
                                         func=AF.Copy)

            st["diffbf"] = diffbf
            return st

        def stage_c(st):
            """Mix + store."""
            xn = st["xn"]
            diags = st["diags"]
            coeffs = st["coeffs"]
            diffbf = st["diffbf"]
            rows = st["rows"]
            t = st["t"]

            mix_pe = st["mix_pe"]
            acts = st["acts"]
            tag_sfx = "L" if len(mix_pe) == N else ""
            mdiags = None
            if mix_pe:
                mpool = diagl if tag_sfx else diagp
                mdiags = mpool.tile([P, 4 * len(mix_pe), P], BF16,
                                    tag=f"mdiags{tag_sfx}", name=f"mdg{t}")
                for i, n in enumerate(mix_pe):
                    for m in range(N):
                        nc.vector.tensor_scalar_mul(
                            mdiags[:, 4 * i + m, :], idbf_s[:],
                            coeffs[:, 4 * m + n:4 * m + n + 1],
                        )

            for i, n in enumerate(mix_pe):
                outsb = outp.tile([P, C], BF16, tag="outsb", name=f"ou{t}_{n}")
                for cc in range(4):
                    cs = slice(cc * 512, (cc + 1) * 512)
                    mix_p = ps_mix.tile([P, 512], F32, tag="mix",
                                        name=f"mx{t}_{n}_{cc}")
                    for src in range(N):
                        nc.tensor.matmul(
                            mix_p[:], mdiags[:, 4 * i + src, :],
                            xn[:, src * C + cc * 512: src * C + (cc + 1) * 512],
                            start=(src == 0), stop=(src == 3),
                        )
                    nc.vector.scalar_tensor_tensor(
                        out=outsb[:, cs],
                        in0=diffbf[:, cs],
                        scalar=acts[:, 4 + n:5 + n], in1=mix_p[:],
                        op0=ALU.mult, op1=ALU.add,
                    )
                nc.sync.dma_start(
                    out=out_d[rows, n * C:(n + 1) * C], in_=outsb[:]
                )

            H = C // 2
            for n in range(N):
                if n in mix_pe:
                    continue
                outsb = outp.tile([P, C], BF16, tag="outsb", name=f"ou{t}_{n}")
                for h in range(2):
                    hs = slice(h * H, (h + 1) * H)
                    ts_ = [
                        mxp.tile([P, H], BF16, tag=f"mt{j}",
                                 name=f"mt{t}_{n}_{h}_{j}")
                        for j in range(4)
                    ]
                    td = mxp.tile([P, H], BF16, tag="mtd", name=f"mtd{t}_{n}_{h}")
                    for m in range(N):
                        nc.vector.tensor_scalar_mul(
                            ts_[m][:], xn[:, m * C + h * H: m * C + (h + 1) * H],
                            coeffs[:, 4 * m + n:4 * m + n + 1],
                        )
                    nc.vector.tensor_scalar_mul(
                        td[:], diffbf[:, hs], acts[:, 4 + n:5 + n]
                    )
                    l1 = POOL_L1.get(n, ())
                    eng0 = nc.gpsimd if 0 in l1 else nc.vector
                    eng1 = nc.gpsimd if 1 in l1 else nc.vector
                    eng0.tensor_tensor(out=ts_[0][:], in0=ts_[0][:],
                                       in1=ts_[1][:], op=ALU.add)
                    eng1.tensor_tensor(out=ts_[2][:], in0=ts_[2][:],
                                       in1=ts_[3][:], op=ALU.add)
                    nc.vector.tensor_tensor(out=ts_[0][:], in0=ts_[0][:],
                                            in1=ts_[2][:], op=ALU.add)
                    nc.vector.tensor_tensor(
                        out=outsb[:, h * H:(h + 1) * H],
                        in0=ts_[0][:], in1=td[:], op=ALU.add,
                    )
                nc.sync.dma_start(
                    out=out_d[rows, n * C:(n + 1) * C], in_=outsb[:]
                )

        # ---- software-pipelined emission: stage A runs one tile ahead ----
        pending = None
        wp_chunks = 0
        for rep in range(reps):
            for t in range(nt):
                st = stage_a(t)
                st["mix_pe"] = (
                    MIX_PE_LAST if (rep == reps - 1 and t == nt - 1) else MIX_PE
                )
                # defer the big weight load until after tile 0's loads
                # (single DMA; emitted before its first consumer stage_b(0))
                if rep == 0 and wp_chunks == 0:
                    nc.sync.dma_start(out=wp_s[:], in_=wp_d[:])
                    wp_chunks = 4
                if pending is not None:
                    stage_b(pending)
                    stage_c(pending)
                pending = st
        stage_b(pending)
        stage_c(pending)


def prep_params(inputs):
    """Host-side parameter preprocessing shared by all cores."""
    bf = ml_dtypes.bfloat16
    W_all = np.asarray(inputs["W_all"], np.float32)
    W_layer = np.asarray(inputs["W_layer"], np.float32)
    b_all = np.asarray(inputs["b_all"], np.float32)
    b_layer = np.asarray(inputs["b_layer"], np.float32)
    perm_mat = np.asarray(inputs["perm_mat"], np.float32)
    a_pre = float(np.asarray(inputs["alpha_pre"]).reshape(-1)[0])
    a_post = float(np.asarray(inputs["alpha_post"]).reshape(-1)[0])
    a_res = float(np.asarray(inputs["alpha_res"]).reshape(-1)[0])

    f8 = ml_dtypes.float8_e4m3
    wallt = np.ascontiguousarray(
        (W_all * 16.0).T.astype(f8).reshape(NK, P, 32).transpose(1, 0, 2)
    )
    wp = (np.ascontiguousarray(W_layer.T) - np.eye(C, dtype=np.float32))
    wp = np.ascontiguousarray(wp.astype(bf).reshape(16, P, C).transpose(1, 0, 2))
    blayer = b_layer.astype(bf).reshape(1, C)
    ones1 = np.ones((1, P), dtype=bf)
    # perm_aug columns m-major: col 4m+n = perm_mat[:, n*4+m]; col 16 = 1
    perm_aug = np.zeros((NFACT, 17), np.float32)
    perm_aug[:, :16] = perm_mat.reshape(NFACT, N, N).transpose(0, 2, 1).reshape(
        NFACT, 16
    )
    perm_aug[:, 16] = 1.0
    alphab = np.zeros((2, 32), np.float32)
    alphab[0, 0:4] = a_pre / 16.0
    alphab[0, 4:8] = a_post / 16.0
    alphab[0, 8:32] = a_res / 16.0
    alphab[1, 0:4] = b_all[0:4]
    alphab[1, 4:8] = b_all[4:8]
    alphab[1, 8:32] = b_all[8:32]
    # cubic activation polys: cols 0:4 sigmoid, 4:8 2*sigmoid, 8:32 exp
    polyc = np.zeros((4, 32), np.float32)
    polyc[0, 0:4], polyc[1, 0:4], polyc[2, 0:4], polyc[3, 0:4] = \
        0.5, 0.25, 0.0, -1.0 / 48.0
    polyc[0, 4:8], polyc[1, 4:8], polyc[2, 4:8], polyc[3, 4:8] = \
        1.0, 0.5, 0.0, -1.0 / 24.0
    polyc[0, 8:32], polyc[1, 8:32], polyc[2, 8:32], polyc[3, 8:32] = \
        1.0, 1.0, 0.5, 1.0 / 6.0
    idbf = np.eye(P, dtype=np.float32).astype(bf)
    idf32 = np.eye(P, dtype=np.float32)
    return {
        "wallt": wallt, "wp": wp, "blayer": blayer, "ones1": ones1,
        "permaug": perm_aug, "alphab": alphab, "polyc": polyc,
        "idbf": idbf, "idf32": idf32,
    }


_PROGRAM_CACHE = {}


def get_program(t_core, with_bias=True):
    key = (t_core, with_bias)
    if key not in _PROGRAM_CACHE:
        _PROGRAM_CACHE[key] = build_program(t_core, with_bias=with_bias)
    return _PROGRAM_CACHE[key]


def run(inputs, trace=False):
    bf = ml_dtypes.bfloat16
    x = np.asarray(inputs["x_streams"], np.float32).reshape(T, NCF).astype(bf)
    params = prep_params(inputs)
    t_core = T // NCORES
    nt = t_core // P
    nc = get_program(t_core, True)
    in_maps = []
    for c in range(NCORES):
        m = dict(params)
        xc = x[c * t_core:(c + 1) * t_core]
        m["x"] = np.ascontiguousarray(xc)
        m["xT"] = np.ascontiguousarray(
            xc.reshape(nt, P, 4, NK // 4, P).transpose(0, 2, 4, 3, 1)
        ).astype(ml_dtypes.float8_e4m3).reshape(nt * 4, P, NK // 4, P)
        in_maps.append(m)
    res = bass_utils.run_bass_kernel_spmd(
        nc, in_maps, core_ids=list(range(NCORES)), trace=trace
    )
    out = np.concatenate([r["out"] for r in res.results], axis=0)
    return out.reshape(T, N, C).astype(np.float32), res


def kernel(**inputs) -> np.ndarray:
    out, _ = run(inputs)
    return out
